# revision 38
# baseline (speedup 1.0000x reference)
"""DialogueGCN windowed-attention relational GCN on 8 Trainium2 NeuronCores.

Sharding: utterance axis N=16384 split into 8 shards of 2048 rows; each core
gets its shard plus a 128-row halo on each side (zero-padded at the global
edges). The small DxD weights are replicated. No collectives needed.

Per-core algorithm (banded ops as dense matmuls over a 2x128-row jj-window
per 128-row output block; supports live on a 64-row-shifted chunk grid so
each block's 255-row band is exactly two aligned support chunks):
  xT    = x_halo^T                          (PE transposes, f32r)
  qT    = (x @ W_att)^T                     (via W_att chunks vs xT)
  S[r]  = x_halo @ Wr_combined, r in a,b,c  (3 supports via mask linearity)
  per block b:
    R[nl, jj] = sum_d qT[d, n] xT[d, j]     (attention logits, [128, 256])
    E    = exp(R + band - max), esum        (ScalarE, accum_out)
    Et   = E^T                              (PE transpose pair -> one PSUM)
    C2   = Et * suc_mask   (POOL)           direction strip
    C3   = Et * same_spk   (DVE stt)        speaker strips
    h    = 6 accumulating matmuls strips^T-contract S[..]
    hs/negmax2/s2 staged; log_softmax finalized in a tail phase so ScalarE
    runs a single activation table (Exp) inside the loop.
"""

import numpy as np

N_TOT, D, W, SPK = 16384, 256, 64, 8
NCORES = 8
NC_ROWS = N_TOT // NCORES          # 2048 rows per core
HALO = 128
NH = NC_ROWS + 2 * HALO            # 2304 rows with halo
NBLK = NC_ROWS // 128              # 16 output blocks per core
NCH = NH // 128                    # 18 halo chunks (aligned grid)
NSH = NCH - 1                      # 17 chunks on the 64-shifted grid
NEG_BIG = -1.0e30

_cache = {}


def _build_bass():
    import concourse.tile as tile
    from concourse import bacc, mybir

    f32 = mybir.dt.float32
    f32r = mybir.dt.float32r
    AX = mybir.AxisListType.X
    OP = mybir.AluOpType
    AF = mybir.ActivationFunctionType

    nc = bacc.Bacc("TRN2", target_bir_lowering=False, debug=False,
                   num_devices=NCORES)

    xh_d = nc.dram_tensor("xh", [NH, D], f32, kind="ExternalInput").ap()
    spk_d = nc.dram_tensor("spk", [NH], f32, kind="ExternalInput").ap()
    wq_d = nc.dram_tensor("wq", [D, D], f32, kind="ExternalInput").ap()
    wa_d = nc.dram_tensor("wa", [D, D], f32, kind="ExternalInput").ap()
    wb_d = nc.dram_tensor("wb", [D, D], f32, kind="ExternalInput").ap()
    wc_d = nc.dram_tensor("wc", [D, D], f32, kind="ExternalInput").ap()
    band_d = nc.dram_tensor("band", [128, 256], f32, kind="ExternalInput").ap()
    sucm_d = nc.dram_tensor("sucm", [128, 256], f32, kind="ExternalInput").ap()
    ident_d = nc.dram_tensor("ident", [128, 128], f32, kind="ExternalInput").ap()
    out_d = nc.dram_tensor("out", [NC_ROWS, D], f32, kind="ExternalOutput").ap()

    with tile.TileContext(nc) as tc:
        from contextlib import ExitStack
        with ExitStack() as ctx:
            const = ctx.enter_context(tc.tile_pool(name="const", bufs=1))
            persist = ctx.enter_context(tc.tile_pool(name="persist", bufs=1))
            work = ctx.enter_context(tc.tile_pool(name="work", bufs=3))
            psum = ctx.enter_context(tc.tile_pool(name="psum", bufs=2, space="PSUM"))

            # ---- constants ----
            band_sb = const.tile([128, 256], f32)
            nc.sync.dma_start(band_sb, band_d)
            zeros_sb = const.tile([128, 256], f32)
            nc.gpsimd.memset(zeros_sb, 0.0)
            sucm_sb = const.tile([128, 256], f32)
            nc.sync.dma_start(sucm_sb, sucm_d)
            ident_sb = const.tile([128, 128], f32)
            nc.sync.dma_start(ident_sb, ident_d)
            ident_r = const.tile([128, 128], f32r)
            nc.vector.tensor_copy(ident_r, ident_sb)

            # weights: [128, k, 256] with k = e-chunk, rounded to f32r
            w_r = {}
            for name, wd in (("wq", wq_d), ("wa", wa_d), ("wb", wb_d), ("wc", wc_d)):
                stage = work.tile([128, 2, D], f32, tag="wstage")
                nc.sync.dma_start(stage, wd.rearrange("(k p) d -> p k d", p=128))
                wr = const.tile([128, 2, D], f32r, name=f"{name}_r")
                nc.vector.tensor_copy(wr, stage)
                w_r[name] = wr

            # speakers: shifted column layout [128, NSH] and broadcast rows
            spk_col = persist.tile([128, NSH], f32)
            nc.sync.dma_start(
                spk_col, spk_d[64:64 + NSH * 128].rearrange("(c p) -> p c", p=128))
            spk_row = persist.tile([1, NC_ROWS], f32)
            nc.sync.dma_start(
                spk_row, spk_d.rearrange("(a b) -> a b", a=1)[:, HALO:HALO + NC_ROWS])
            spk_bc = persist.tile([128, NC_ROWS], f32)
            nc.gpsimd.partition_broadcast(spk_bc, spk_row)

            # ---- xT: transposed halo embeddings [128, 2(d-half), NH] f32r ----
            # xh arrives in 3 batched DMAs; per chunk-pair, 4 PE transposes
            # fill one PSUM bank, drained by a single [128, 512] ACT copy.
            xT = persist.tile([128, 2, NH], f32r)
            xh_sb = persist.tile([128, NCH, D], f32)
            xh_v = xh_d.rearrange("(c p) d -> p c d", p=128)
            for g in range(NCH // 2):
                nc.sync.dma_start(xh_sb[:, g * 2:(g + 1) * 2, :],
                                  xh_v[:, g * 2:(g + 1) * 2, :])
            for c2 in range(NCH // 2):
                ptx = psum.tile([128, 512], f32, tag="ph")
                for i in (0, 1):
                    for k in (0, 1):
                        nc.tensor.transpose(
                            ptx[:, k * 256 + i * 128: k * 256 + (i + 1) * 128],
                            xh_sb[:, 2 * c2 + i, k * 128:(k + 1) * 128], ident_sb)
                nc.scalar.copy(
                    xT[:, :, 2 * c2 * 128:(2 * c2 + 2) * 128],
                    ptx.rearrange("p (k n) -> p k n", k=2))

            qT = persist.tile([128, 2, NC_ROWS], f32r)
            S = persist.tile([128, 3, NSH, D], f32r)

            # ---- staging for the log_softmax tail ----
            hs_all = persist.tile([128, NBLK, D], f32)
            negmax2_all = persist.tile([128, NBLK], f32)
            s2_all = persist.tile([128, NBLK], f32)
            rinv_all = persist.tile([128, NBLK], f32)

            # ---- qT: one 512-column group ----
            def emit_qT(g):
                nsl = slice(HALO + g * 512, HALO + (g + 1) * 512)
                for dh in (0, 1):
                    psq = psum.tile([128, 512], f32, tag="ph", name="psq")
                    for k in (0, 1):
                        nc.tensor.matmul(
                            psq, w_r["wq"][:, k, dh * 128:(dh + 1) * 128],
                            xT[:, k, nsl], start=(k == 0), stop=(k == 1))
                    nc.scalar.copy(qT[:, dh, g * 512:(g + 1) * 512], psq)

            # ---- one support chunk on the 64-shifted grid ----
            def emit_S(c):
                csl = slice(64 + c * 128, 64 + (c + 1) * 128)
                pab = psum.tile([128, 512], f32, tag="ph", name="pab")
                for i, name in enumerate(("wa", "wb")):
                    for k in (0, 1):
                        nc.tensor.matmul(
                            pab[:, i * 256:(i + 1) * 256], xT[:, k, csl],
                            w_r[name][:, k, :], start=(k == 0), stop=(k == 1))
                pab_v = pab.rearrange("p (i d) -> p i d", i=2)
                if c % 2 == 0:
                    nc.scalar.copy(S[:, 0:2, c, :], pab_v)
                else:
                    nc.vector.tensor_copy(S[:, 0:2, c, :], pab_v)
                pwc = psum.tile([128, D], f32, tag="ph", name="pwc")
                for k in (0, 1):
                    nc.tensor.matmul(pwc, xT[:, k, csl], w_r["wc"][:, k, :],
                                     start=(k == 0), stop=(k == 1))
                nc.vector.tensor_copy(S[:, 2, c, :], pwc)

            # ---- one 128-row output block ----
            def emit_block(b):
                nsl = slice(b * 128, (b + 1) * 128)
                # attention logits R [128, 256]: jj-window = halo cols
                # [b*128+64, b*128+320)
                psr = psum.tile([128, 256], f32, tag="psr")
                for k in (0, 1):
                    nc.tensor.matmul(psr, qT[:, k, nsl],
                                     xT[:, k, b * 128 + 64: b * 128 + 320],
                                     start=(k == 0), stop=(k == 1))
                # rm = R + band (band = -1e30 out-of-band); e = exp(rm - max)
                rm = work.tile([128, 256], f32, tag="rm")
                nc.vector.tensor_tensor(rm, psr, band_sb, op=OP.add)
                negmax = work.tile([128, 1], f32, tag="negmax")
                nc.vector.reduce_max(negmax, rm, axis=AX, negate=True)
                ee = work.tile([128, 256], f32r, tag="ee")
                esum = work.tile([128, 1], f32, tag="esum")
                nc.scalar.activation(ee, rm, AF.Exp, bias=negmax,
                                     accum_out=esum)
                rinv = rinv_all[:, b:b + 1]
                nc.vector.reciprocal(rinv, esum)

                # transposed strip pair Et [128, 256] (cols 0:128 = chunk A)
                pte = psum.tile([128, 256], f32r, tag="pte")
                for c in (0, 1):
                    nc.tensor.transpose(pte[:, c * 128:(c + 1) * 128],
                                        ee[:, c * 128:(c + 1) * 128], ident_r)
                et = work.tile([128, 256], f32r, tag="et")
                if b % 2 == 0:
                    nc.scalar.copy(et, pte)
                else:
                    nc.vector.tensor_copy(et, pte)

                # direction strip (POOL) and same-speaker strips (POOL + DVE)
                c2 = work.tile([128, 256], f32r, tag="c2")
                nc.gpsimd.tensor_tensor(c2, et, sucm_sb, op=OP.mult)
                c3 = work.tile([128, 256], f32r, tag="c3")
                for c, eng in ((0, nc.vector), (1, nc.vector)):
                    eng.scalar_tensor_tensor(
                        c3[:, c * 128:(c + 1) * 128],
                        in0=spk_bc[:, nsl], scalar=spk_col[:, b + c:b + c + 1],
                        in1=et[:, c * 128:(c + 1) * 128],
                        op0=OP.is_equal, op1=OP.mult)

                # aggregation: 6 accumulating banded matmuls
                psh = psum.tile([128, D], f32, tag="psh")
                mms = [(et, 0, 0), (et, 1, 0), (c2, 0, 1), (c2, 1, 1),
                       (c3, 0, 2), (c3, 1, 2)]
                for i, (strip, c, r) in enumerate(mms):
                    nc.tensor.matmul(psh, strip[:, c * 128:(c + 1) * 128],
                                     S[:, r, b + c, :],
                                     start=(i == 0), stop=(i == len(mms) - 1))

                # stage raw h; no max-centering needed for the d-softmax
                # (|h * rinv| is bounded far below exp overflow)
                nc.vector.tensor_copy(hs_all[:, b, :], psh)
                e2 = work.tile([128, D], f32, tag="e2")
                nc.scalar.activation(e2, hs_all[:, b, :], AF.Exp,
                                     scale=rinv,
                                     accum_out=s2_all[:, b:b + 1])

                # finalize a group of 4 blocks as soon as it completes (Ln and
                # Identity share the Exp activation table set -> no reloads)
                if b % 4 == 3:
                    g = b // 4
                    gs = slice(g * 4, g * 4 + 4)
                    ln4 = work.tile([128, 4], f32, tag="ln4")
                    nc.scalar.activation(ln4, s2_all[:, gs], AF.Ln)
                    bias4 = work.tile([128, 4], f32, tag="bias4")
                    nc.vector.tensor_scalar_mul(bias4, ln4, -1.0)
                    ob4 = work.tile([128, 4, D], f32, tag="ob4")
                    for i in range(4):
                        bb = 4 * g + i
                        nc.vector.tensor_scalar(
                            ob4[:, i, :], hs_all[:, bb, :],
                            scalar1=rinv_all[:, bb:bb + 1],
                            scalar2=bias4[:, i:i + 1],
                            op0=OP.mult, op1=OP.add)
                    nc.sync.dma_start(
                        out_d.rearrange("(c p) d -> p c d", p=128)[:, gs, :], ob4)

            # ---- interleaved driver: mix GEMM phases with block groups so
            # every scheduling window has PE, DVE, ACT, and POOL work ----
            s_next = 0
            for g in range(NBLK // 4):
                emit_qT(g)
                hi = min(4 * (g + 1) + 1, NSH)
                while s_next < hi:
                    emit_S(s_next)
                    s_next += 1
                for i in range(4):
                    emit_block(4 * g + i)

    nc.compile()
    return nc


def _host_constants():
    nl = np.arange(128)[:, None]
    jj = np.arange(256)[None, :]
    band = np.where((jj >= nl) & (jj < nl + 128), 0.0, NEG_BIG).astype(np.float32)
    p = np.arange(128)[:, None]
    f = np.arange(128)[None, :]
    suc = np.concatenate([(p < f + 64), (p < f - 64)], axis=1).astype(np.float32)
    ident = np.eye(128, dtype=np.float32)
    return band, suc, ident


def _prep_in_maps(np_inputs):
    x = np.asarray(np_inputs["x"], dtype=np.float32)
    spk = np.asarray(np_inputs["speaker_ids"]).astype(np.float32)
    W_att = np.asarray(np_inputs["W_att"], dtype=np.float32)
    W_pred = np.asarray(np_inputs["W_pred"], dtype=np.float32)
    W_suc = np.asarray(np_inputs["W_suc"], dtype=np.float32)
    W_same = np.asarray(np_inputs["W_same"], dtype=np.float32)
    W_diff = np.asarray(np_inputs["W_diff"], dtype=np.float32)

    band, sucm, ident = _host_constants()
    wa = W_pred + W_diff
    wb = W_suc - W_pred
    wc = W_same - W_diff

    xp = np.zeros((N_TOT + 2 * HALO, D), dtype=np.float32)
    xp[HALO:HALO + N_TOT] = x
    spkp = np.full((N_TOT + 2 * HALO,), -1.0, dtype=np.float32)
    spkp[HALO:HALO + N_TOT] = spk

    in_maps = []
    for k in range(NCORES):
        r0 = k * NC_ROWS
        in_maps.append({
            "xh": np.ascontiguousarray(xp[r0:r0 + NH]),
            "spk": np.ascontiguousarray(spkp[r0:r0 + NH]),
            "wq": W_att, "wa": wa, "wb": wb, "wc": wc,
            "band": band, "sucm": sucm, "ident": ident,
        })
    return in_maps


def kernel(x, speaker_ids, W_att, W_pred, W_suc, W_same, W_diff):
    from concourse import bass_utils

    if "nc" not in _cache:
        _cache["nc"] = _build_bass()
    nc = _cache["nc"]

    in_maps = _prep_in_maps({
        "x": x, "speaker_ids": speaker_ids, "W_att": W_att, "W_pred": W_pred,
        "W_suc": W_suc, "W_same": W_same, "W_diff": W_diff})

    res = bass_utils.run_bass_kernel_spmd(nc, in_maps, core_ids=list(range(NCORES)))
    _cache["last_result"] = res
    return np.concatenate([res.results[k]["out"] for k in range(NCORES)], axis=0)
